# revision 1
# baseline (speedup 1.0000x reference)
"""AttnRNN decoder kernel for trn2 (8 NeuronCores, data-parallel over batch).

Structure:
  host   : embedding gather, weight transposes, batch sharding (B=32 -> 4/core)
  device : phase0  seq_qT / keyT_enc precompute (t-independent attention terms)
           phase1  127 sequential attention+LSTM steps, small-state column form
           phase2  hoisted vocab projection (bf16) + log_softmax + DMA out
"""

import numpy as np

import concourse.bass as bass
import concourse.bacc as bacc
import concourse.mybir as mybir
import concourse.tile as tile
from concourse.bass_utils import run_bass_kernel_spmd

F32 = mybir.dt.float32
F32R = mybir.dt.float32r
BF16 = mybir.dt.bfloat16
AF = mybir.ActivationFunctionType

B, L, D, T = 32, 512, 256, 128
H, NCOM, NB = 256, 8000, 128
V = NCOM + NB
NCORES = 8
BS = B // NCORES          # 4 examples per core
TT = T - 1                # 127 decode steps

# vocab chunking for phase 2: 15x512 + 1x320 common, then 128 batched
VCH = [(0, 512)] * 0 + [(i * 512, 512) for i in range(15)] + [(7680, 320)]


def _r(ap):
    return ap


def _bf(ap):
    return ap


def _pmajor(x, nchunk):
    """(nchunk*128, ...) -> (128, nchunk, ...) partition-major."""
    s = x.shape
    return np.ascontiguousarray(
        x.reshape(nchunk, 128, *s[1:]).transpose(1, 0, *range(2, 1 + len(s)))
    )


def _build(cfg):
    """Build the single-core program (SPMD-replicated across 8 cores)."""
    nc = bacc.Bacc("TRN2", target_bir_lowering=False, debug=False)

    dr = {}

    def din(name, shape, dt=F32):
        dr[name] = nc.dram_tensor(name, list(shape), dt, kind="ExternalInput").ap()
        return dr[name]

    enc_r = din("enc_r", (128, BS, 4, D), BF16)    # enc[b, 128*lc+p, d]
    enct = din("enct", (128, 2, BS, L), BF16)      # enc[b, l, 128*dc+p]
    tgtt = din("tgtt", (128, 2, TT, BS), BF16)     # tgt[b, t, 128*dc+p]
    cembt = din("cembt", (128, 2, NCOM), BF16)     # common[v, 128*dc+p]
    obt = din("obt", (128, 2, BS, NB), BF16)       # batched[b, v, 128*dc+p]
    wqt = din("wqt", (128, 2, D), BF16)                  # Wq[e, 128*dc+p]
    wket = din("wket", (128, 2, D), BF16)
    wkht = din("wkht", (128, 2, D), BF16)
    wkh2 = din("wkh2", (128, 2, D), BF16)   # akw[e, 256+f] with e on partitions
    cwt = din("cwt", (128, 4, H), BF16)                  # combine_w[g, 128*fc+p]
    lwt = din("lwt", (128, 4, 4 * H), BF16)              # [W_ih.T ; W_hh.T]
    owt = din("owt", (128, 2, D), BF16)
    idn = din("idn", (128, 128))
    if cfg["any_bias"]:
        bqc = din("bqc", (128, 2))
        bkc = din("bkc", (128, 2))
        brow = din("brow", (1, 2 * H + 4 * H + D), BF16)   # [bc(256), bl(1024), bo(256)]
        ones = din("ones", (1, BS), BF16)
    if cfg["enc_mask"]:
        emadd = din("emadd", (97, L))
    if cfg["out_mask"]:
        bmr = din("bmr", (BS, NB), BF16)
        onest = din("onest", (1, TT), BF16)

    out = nc.dram_tensor("out", [BS, TT, V], F32, kind="ExternalOutput").ap()

    with tile.TileContext(nc) as tc:
        with (
            tc.tile_pool(name="const", bufs=1) as kc,
            tc.tile_pool(name="state", bufs=3) as stp,
        ):
            # ---- persistent SBUF loads ----
            enc_sb = kc.tile([128, BS, 4, D], BF16)
            nc.sync.dma_start(enc_sb[:], enc_r[:])
            tgtt_sb = kc.tile([128, 2, TT, BS], BF16)
            nc.sync.dma_start(tgtt_sb[:], tgtt[:])
            wqt_sb = kc.tile([128, 2, D], BF16)
            nc.sync.dma_start(wqt_sb[:], wqt[:])
            wket_sb = kc.tile([128, 2, D], BF16)
            nc.sync.dma_start(wket_sb[:], wket[:])
            wkht_sb = kc.tile([128, 2, D], BF16)
            nc.sync.dma_start(wkht_sb[:], wkht[:])
            wkh2_sb = kc.tile([128, 2, D], BF16)
            nc.sync.dma_start(wkh2_sb[:], wkh2[:])
            cwt_sb = kc.tile([128, 4, H], BF16)
            nc.sync.dma_start(cwt_sb[:], cwt[:])
            lwt_sb = kc.tile([128, 4, 4 * H], BF16)
            nc.sync.dma_start(lwt_sb[:], lwt[:])
            owt_sb = kc.tile([128, 2, D], BF16)
            nc.sync.dma_start(owt_sb[:], owt[:])
            idn_sb = kc.tile([128, 128], F32)
            nc.sync.dma_start(idn_sb[:], idn[:])
            if cfg["any_bias"]:
                bqc_sb = kc.tile([128, 2], F32)
                nc.sync.dma_start(bqc_sb[:], bqc[:])
                bkc_sb = kc.tile([128, 2], F32)
                nc.sync.dma_start(bkc_sb[:], bkc[:])
                brow_sb = kc.tile([1, 2 * H + 4 * H + D], BF16)
                nc.sync.dma_start(brow_sb[:], brow[:])
                ones_sb = kc.tile([1, BS], BF16)
                nc.sync.dma_start(ones_sb[:], ones[:])
            if cfg["enc_mask"]:
                emadd_sb = kc.tile([97, L], F32)
                nc.sync.dma_start(emadd_sb[:], emadd[:])
            if cfg["out_mask"]:
                bmr_sb = kc.tile([BS, NB], BF16)
                nc.sync.dma_start(bmr_sb[:], bmr[:])
                onest_sb = kc.tile([1, TT], BF16)
                nc.sync.dma_start(onest_sb[:], onest[:])

            # phase-2-only tensors: issue their (large) DMAs last so they
            # don't delay the loads that gate phase 0/1 startup
            cembt_sb = kc.tile([128, 2, NCOM], BF16)
            nc.sync.dma_start(cembt_sb[:], cembt[:])
            obt_sb = kc.tile([128, 2, BS, NB], BF16)
            nc.sync.dma_start(obt_sb[:], obt[:])

            seqqt_sb = kc.tile([128, 2, BS, L], BF16)
            m2t_sb = kc.tile([128, 2, BS, L], BF16)
            ket_sb = kc.tile([128, 2, TT, BS], BF16)
            linT_sb = kc.tile([128, 2, BS, TT], BF16)

            # ---- phase 0: seq_qT and keyT_enc ----
            with (
                tc.tile_pool(name="enctp", bufs=1) as ep,
                tc.tile_pool(name="p0ps", bufs=2, space="PSUM") as p0,
            ):
                enct_sb = ep.tile([128, 2, BS, L], BF16)
                nc.sync.dma_start(enct_sb[:], enct[:])
                for b in range(BS):
                    for c in range(2):
                        ps = p0.tile([128, 512], F32)
                        for k in range(2):
                            nc.tensor.matmul(
                                ps[:],
                                _r(wqt_sb[:, k, c * 128:(c + 1) * 128]),
                                _r(enct_sb[:, k, b, :]),
                                start=(k == 0), stop=(k == 1),
                            )
                        if cfg["any_bias"]:
                            nc.scalar.activation(
                                seqqt_sb[:, c, b, :], ps[:], AF.Identity,
                                bias=bqc_sb[:, c:c + 1],
                            )
                        else:
                            nc.vector.tensor_copy(seqqt_sb[:, c, b, :], ps[:])
                for b in range(BS):
                    for c in range(2):
                        ps = p0.tile([128, 512], F32)
                        for k in range(2):
                            nc.tensor.matmul(
                                ps[:],
                                _r(wkh2_sb[:, k, c * 128:(c + 1) * 128]),
                                _r(seqqt_sb[:, k, b, :]),
                                start=(k == 0), stop=(k == 1),
                            )
                        nc.vector.tensor_copy(m2t_sb[:, c, b, :], ps[:])
                for c in range(2):
                    ps = p0.tile([128, 512], F32)
                    for k in range(2):
                        nc.tensor.matmul(
                            ps[:, 0:TT * BS],
                            _r(wket_sb[:, k, c * 128:(c + 1) * 128]),
                            _r(tgtt_sb[:, k, :, :]),
                            start=(k == 0), stop=(k == 1),
                        )
                    if cfg["any_bias"]:
                        nc.scalar.activation(
                            ket_sb[:, c, :, :], ps[:, 0:TT * BS], AF.Identity,
                            bias=bkc_sb[:, c:c + 1],
                        )
                    else:
                        nc.vector.tensor_copy(ket_sb[:, c, :, :], ps[:, 0:TT * BS])

            # ---- phase 1: 127 sequential steps ----
            # examples live at partition strips 32*b for per-example matmul
            # outputs (PE col-group tiling); compacted back via ::32 APs.
            P97 = 97
            with (
                tc.tile_pool(name="sps", bufs=1, space="PSUM") as sps,
                tc.tile_pool(name="khp", bufs=1, space="PSUM") as khp,
                tc.tile_pool(name="sclp", bufs=1, space="PSUM") as sclp,
                tc.tile_pool(name="medps", bufs=1, space="PSUM") as medps,
                tc.tile_pool(name="gps", bufs=1, space="PSUM") as gps,
                tc.tile_pool(name="work", bufs=3) as sbw,
            ):
                # persistent psum scratch (memset once; matmuls only touch
                # rows 32*b each step, engines read the zero rows harmlessly):
                # scl (128,768): scores [0:97,0:512], lin [0:4,512:768]
                # mp  (97,512): attn [0:97,0:256], comb [0:4,256:512]
                scl = sclp.tile([128, 768], F32, tag="scl")
                nc.vector.memset(scl[:], 0.0)
                mp = medps.tile([P97, 512], F32, tag="mp")
                nc.vector.memset(mp[:], 0.0)
                hT_cur = None
                c_cur = None
                for t in range(0 if cfg.get("skip_p1") else TT):
                    # psum scratch:
                    # spw (128,6,97): wT [0:4], atT [4:6]
                    # kh  (128,8,4): kh [0:2], cbT [2:4], hT [4:6], lT [6:8]
                    spw = sps.tile([128, 6, 128], F32, tag="spw")
                    kh = khp.tile([128, 8, BS], F32, tag="kh")

                    # -- scores: enc-key half is h-independent (runs
                    #    early); h half (hT @ M2T) is the critical path --
                    for b in range(BS):
                        for k in range(2):
                            nc.tensor.matmul(
                                scl[32 * b:32 * b + 1, 0:L],
                                _r(ket_sb[:, k, t, b:b + 1]),
                                _r(seqqt_sb[:, k, b, :]),
                                start=(k == 0), stop=(k == 1 and t == 0),
                                tile_position=(0, 32 * b),
                            )
                    if t > 0:
                        for b in range(BS):
                            for k in range(2):
                                nc.tensor.matmul(
                                    scl[32 * b:32 * b + 1, 0:L],
                                    _r(hT_cur[:, k, b:b + 1]),
                                    _r(m2t_sb[:, k, b, :]),
                                    start=False, stop=(k == 1),
                                    tile_position=(0, 32 * b),
                                )
                    if cfg["enc_mask"]:
                        nc.vector.tensor_add(
                            scl[0:P97, 0:L], scl[0:P97, 0:L], emadd_sb[:]
                        )

                    if cfg.get("abl_attn"):
                        atT_f = [ktv[0], ktv[1]]
                    else:
                        atT_f = None
                    # -- softmax (unnormalized exp + row sum; no max shift) --
                    ex = sbw.tile([P97, L], F32, tag="ex")
                    if atT_f is None:
                        s_ = sbw.tile([P97, 1], F32, tag="s")
                        nc.scalar.activation(
                            ex[:], scl[0:P97, 0:L], AF.Exp, accum_out=s_[:]
                        )
                        r_ = sbw.tile([P97, 1], F32, tag="r")
                        nc.vector.reciprocal(r_[:], s_[:])

                        # -- transpose exp(scores) to column form --
                        for j in range(4):
                            nc.tensor.transpose(
                                _r(spw[:, j, 0:BS]),
                                _r(ex[:, j * 128:(j + 1) * 128]),
                                _r(idn_sb[0:P97, 0:128:32]),
                            )
                        # only columns {0,32,64,96} (the real examples)
                        # are consumed downstream - copy just those
                        wt = sbw.tile([128, 4, BS], BF16, tag="wt")
                        nc.vector.tensor_copy(
                            wt[:], spw[:, 0:4, 0:BS]
                        )

                        # -- attention context (per-example matvec over enc) --
                        for b in range(BS):
                            for k in range(4):
                                nc.tensor.matmul(
                                    mp[32 * b:32 * b + 1, 0:D],
                                    _r(wt[:, k, b:b + 1]),
                                    _r(enc_sb[:, b, k, :]),
                                    start=(k == 0), stop=(k == 3),
                                    tile_position=(0, 32 * b),
                                )
                        at = sbw.tile([P97, D], F32, tag="at")
                        nc.vector.tensor_scalar_mul(at[:], mp[0:P97, 0:D], r_[:])
                        for j in range(2):
                            nc.tensor.transpose(
                                _r(spw[:, 4 + j, 0:BS]),
                                _r(at[:, j * 128:(j + 1) * 128]),
                                _r(idn_sb[0:P97, 0:128:32]),
                            )
                        atT = sbw.tile([128, 2, BS], BF16, tag="atT")
                        nc.vector.tensor_copy(
                            atT[:], spw[:, 4:6, 0:BS]
                        )

                        atT_f = [atT[:, 0, :], atT[:, 1, :]]
                    # -- combine + relu --
                    cl = [tgtt_sb[:, 0, t, :], tgtt_sb[:, 1, t, :],
                          atT_f[0], atT_f[1]]
                    nb_ = 1 if cfg["any_bias"] else 0
                    for k in range(4):
                        nc.tensor.matmul(
                            mp[0:BS, D:2 * D], _r(cl[k]), _r(cwt_sb[:, k, :]),
                            start=(k == 0), stop=(k == 3 and nb_ == 0),
                        )
                    if cfg["any_bias"]:
                        nc.tensor.matmul(
                            mp[0:BS, D:2 * D], _r(ones_sb[0:1, :]),
                            _r(brow_sb[0:1, 0:H]),
                            start=False, stop=True,
                        )
                    cb = sbw.tile([BS, H], F32, tag="cb")
                    nc.scalar.activation(cb[:], mp[0:BS, D:2 * D], AF.Relu)
                    for j in range(2):
                        nc.tensor.transpose(
                            _r(kh[:, 2 + j, :]),
                            _r(cb[:, j * 128:(j + 1) * 128]),
                            _r(idn_sb[0:BS, 0:BS]),
                        )
                    cbT = sbw.tile([128, 2, BS], BF16, tag="cbT")
                    nc.vector.tensor_copy(cbT[:], kh[:, 2:4, :])

                    # -- LSTM gates --
                    gp = gps.tile([BS, 4 * H], F32, tag="gp")
                    gl = []
                    if t > 0:
                        gl += [(hT_cur[:, 0, :], 2), (hT_cur[:, 1, :], 3)]
                    gl += [(cbT[:, 0, :], 0), (cbT[:, 1, :], 1)]
                    for n in range(2):
                        for i, (lt, ki) in enumerate(gl):
                            nc.tensor.matmul(
                                gp[:, n * 512:(n + 1) * 512],
                                _r(lt),
                                _r(lwt_sb[:, ki, n * 512:(n + 1) * 512]),
                                start=(i == 0),
                                stop=(i == len(gl) - 1 and nb_ == 0),
                            )
                        if cfg["any_bias"]:
                            nc.tensor.matmul(
                                gp[:, n * 512:(n + 1) * 512],
                                _r(ones_sb[0:1, :]),
                                _r(brow_sb[0:1, H + n * 512:H + (n + 1) * 512]),
                                start=False, stop=True,
                            )

                    sif = sbw.tile([BS, 512], F32, tag="sif")
                    nc.scalar.activation(sif[:], gp[:, 0:512], AF.Sigmoid)
                    tg = sbw.tile([BS, H], F32, tag="tg")
                    nc.scalar.activation(tg[:], gp[:, 512:768], AF.Tanh)
                    so = sbw.tile([BS, H], F32, tag="so")
                    nc.scalar.activation(so[:], gp[:, 768:1024], AF.Sigmoid)

                    ig = stp.tile([BS, H], F32, tag="cstate")
                    nc.vector.tensor_mul(ig[:], sif[:, 0:H], tg[:])
                    if t > 0:
                        fc_ = sbw.tile([BS, H], F32, tag="fc")
                        nc.vector.tensor_mul(fc_[:], sif[:, H:2 * H], c_cur[:])
                        c_new = stp.tile([BS, H], F32, tag="cstate")
                        nc.vector.tensor_add(c_new[:], ig[:], fc_[:])
                    else:
                        c_new = ig
                    tc_ = sbw.tile([BS, H], F32, tag="tc")
                    nc.scalar.activation(tc_[:], c_new[:], AF.Tanh)
                    hr = sbw.tile([BS, H], F32, tag="hr")
                    nc.vector.tensor_mul(hr[:], so[:], tc_[:])

                    for j in range(2):
                        nc.tensor.transpose(
                            _r(kh[:, 4 + j, :]),
                            _r(hr[:, j * 128:(j + 1) * 128]),
                            _r(idn_sb[0:BS, 0:BS]),
                        )
                    hT_new = stp.tile([128, 2, BS], BF16, tag="hstate")
                    nc.vector.tensor_copy(hT_new[:], kh[:, 4:6, :])

                    # -- output projection, column form: linT = owtT.T@hT --
                    for c in range(2):
                        for k in range(2):
                            nc.tensor.matmul(
                                kh[:, 6 + c, :],
                                _r(owt_sb[:, k, c * 128:(c + 1) * 128]),
                                _r(hT_new[:, k, :]),
                                start=(k == 0), stop=(k == 1),
                            )
                    if cfg["any_bias"]:
                        for c in range(2):
                            nc.tensor.matmul(
                                kh[:, 6 + c, :],
                                _r(brow_sb[0:1, 5 * H + c * 128:5 * H + (c + 1) * 128]),
                                _r(ones_sb[0:1, :]),
                                start=False, stop=True,
                            )
                    nc.vector.tensor_copy(
                        linT_sb[:, :, :, t], kh[:, 6:8, :]
                    )

                    hT_cur, c_cur = hT_new, c_new

            # ---- phase 2: vocab projection + log_softmax ----
            with (
                tc.tile_pool(name="p2ps", bufs=4, space="PSUM") as p2,
                tc.tile_pool(name="ep2", bufs=1) as ep2,
                tc.tile_pool(name="outst", bufs=3) as osp,
                tc.tile_pool(name="sm2", bufs=2) as sm2,
            ):
                nch = len(VCH) + 1
                for b in range(0 if cfg.get("skip_p2") else BS):
                    et = ep2.tile([TT, nch, 512], F32, tag="et")
                    ss = sm2.tile([TT, nch], F32, tag="ss")
                    for j in range(nch):
                        if j < len(VCH):
                            off, w = VCH[j]
                            rhs = [cembt_sb[:, k, off:off + w] for k in range(2)]
                        else:
                            w = NB
                            rhs = [obt_sb[:, k, b, :] for k in range(2)]
                        ps = p2.tile([TT, 512], F32, tag="p2")
                        for k in range(2):
                            nc.tensor.matmul(
                                ps[:, 0:w],
                                linT_sb[:, k, b, :],
                                rhs[k],
                                start=(k == 0),
                                stop=(k == 1 and not (j == nch - 1 and cfg["out_mask"])),
                            )
                        if j == nch - 1 and cfg["out_mask"]:
                            nc.tensor.matmul(
                                ps[:, 0:w], _r(onest_sb[0:1, :]),
                                _r(bmr_sb[b:b + 1, :]),
                                start=False, stop=True,
                            )
                        nc.scalar.activation(
                            et[:, j, 0:w], ps[:, 0:w], AF.Exp,
                            accum_out=ss[:, j:j + 1],
                        )
                    st = sm2.tile([TT, 1], F32, tag="st")
                    nc.vector.reduce_sum(
                        st[:], ss[:], axis=mybir.AxisListType.X
                    )
                    rt = sm2.tile([TT, 1], F32, tag="rt")
                    nc.vector.reciprocal(rt[:], st[:])
                    for j in range(nch):
                        if j < len(VCH):
                            off, w = VCH[j]
                            voff = off
                        else:
                            w, voff = NB, NCOM
                        ot = osp.tile([TT, 512], F32, tag="ot")
                        nc.scalar.activation(
                            ot[:, 0:w], et[:, j, 0:w], AF.Ln, scale=rt[:],
                        )
                        nc.sync.dma_start(
                            out[b, :, voff:voff + w], ot[:, 0:w]
                        )

    nc.compile()
    return nc


_CACHE = {}


def kernel(**inputs):
    inp = {k: np.asarray(v) for k, v in inputs.items()}
    enc = inp["encoder_outputs"].astype(np.float32)
    encm = inp["encoder_outputs_mask"]
    ob = inp["output_batched_encodings"].astype(np.float32)
    obm = inp["output_batched_encodings_mask"]
    idx = inp["target_idxs"]
    cem = inp["common_embedding"].astype(np.float32)
    akw = inp["attn_key_w"].astype(np.float32)
    akb = inp["attn_key_b"].astype(np.float32)
    aqw = inp["attn_query_w"].astype(np.float32)
    aqb = inp["attn_query_b"].astype(np.float32)
    cw = inp["combine_w"].astype(np.float32)
    cb = inp["combine_b"].astype(np.float32)
    wih = inp["lstm_w_ih"].astype(np.float32)
    whh = inp["lstm_w_hh"].astype(np.float32)
    bih = inp["lstm_b_ih"].astype(np.float32)
    bhh = inp["lstm_b_hh"].astype(np.float32)
    ow = inp["out_w"].astype(np.float32)
    obias = inp["out_b"].astype(np.float32)

    # teacher-forced embedding gather (host: data-dependent indexing)
    is_c = idx < NCOM
    cidx = np.clip(idx, 0, NCOM - 1)
    bidx = np.clip(idx - NCOM, 0, NB - 1)
    ge_c = cem[cidx]                                   # (B, T, D)
    ge_b = np.take_along_axis(ob, bidx[..., None], axis=1)
    tgt = np.where(is_c[..., None], ge_c, ge_b)[:, :TT, :].astype(np.float32)

    any_bias = bool(
        np.any(akb) or np.any(aqb) or np.any(cb) or np.any(bih)
        or np.any(bhh) or np.any(obias)
    )
    enc_mask = not bool(encm.all())
    out_mask = not bool(obm.all())

    cfg = {"any_bias": any_bias, "enc_mask": enc_mask, "out_mask": out_mask}
    key = (any_bias, enc_mask, out_mask)
    if key not in _CACHE:
        _CACHE[key] = _build(cfg)
    nc = _CACHE[key]

    # shared (replicated) tensors
    import ml_dtypes
    bft = ml_dtypes.bfloat16
    shared = {
        "cembt": _pmajor(np.ascontiguousarray(cem.T), 2).astype(bft),
        "wqt": _pmajor(np.ascontiguousarray(aqw.T), 2).astype(bft),
        "wket": _pmajor(np.ascontiguousarray(akw[:, :D].T), 2).astype(bft),
        "wkht": _pmajor(np.ascontiguousarray(akw[:, D:].T), 2).astype(bft),
        "wkh2": _pmajor(np.ascontiguousarray(akw[:, D:]), 2).astype(bft),
        "cwt": _pmajor(np.ascontiguousarray(cw.T), 4).astype(bft),
        "lwt": _pmajor(
            np.concatenate([wih.T, whh.T], axis=0), 4
        ).astype(bft),
        "owt": _pmajor(np.ascontiguousarray(ow.T), 2).astype(bft),
        "idn": np.eye(128, dtype=np.float32),
    }
    if any_bias:
        shared["bqc"] = _pmajor(aqb, 2)
        shared["bkc"] = _pmajor(akb, 2)
        shared["brow"] = np.concatenate(
            [cb, bih + bhh, obias]
        )[None, :].astype(bft)
        shared["ones"] = np.ones((1, BS), bft)
    if out_mask:
        shared["onest"] = np.ones((1, TT), bft)

    in_maps = []
    for c in range(NCORES):
        sl = slice(c * BS, (c + 1) * BS)
        e = enc[sl]                                    # (BS, L, D)
        tg_ = tgt[sl]                                  # (BS, TT, D)
        obs = ob[sl]                                   # (BS, NB, D)
        m = dict(shared)
        m["enc_r"] = np.ascontiguousarray(
            e.reshape(BS, 4, 128, D).transpose(2, 0, 1, 3)
        ).astype(bft)
        m["enct"] = np.ascontiguousarray(
            e.transpose(2, 0, 1).reshape(2, 128, BS, L).transpose(1, 0, 2, 3)
        ).astype(bft)
        m["tgtt"] = np.ascontiguousarray(
            tg_.transpose(2, 1, 0).reshape(2, 128, TT, BS).transpose(1, 0, 2, 3)
        ).astype(bft)
        m["obt"] = np.ascontiguousarray(
            obs.transpose(2, 0, 1).reshape(2, 128, BS, NB).transpose(1, 0, 2, 3)
        ).astype(ml_dtypes.bfloat16)
        if enc_mask:
            em97 = np.zeros((97, L), np.float32)
            em97[0:97:32] = np.where(encm[sl], 0.0, -1e30)
            m["emadd"] = em97
        if out_mask:
            m["bmr"] = np.where(obm[sl], 0.0, -1e30).astype(bft)
        in_maps.append(m)

    res = run_bass_kernel_spmd(nc, in_maps, list(range(NCORES)))
    outs = [res.results[c]["out"].reshape(BS, TT, V) for c in range(NCORES)]
    return np.concatenate(outs, axis=0).astype(np.float32)



# revision 3
# speedup vs baseline: 2.0602x; 2.0602x over previous
"""AttnRNN decoder kernel for trn2 (8 NeuronCores, data-parallel over batch).

Structure:
  host   : embedding gather, weight transposes, batch sharding (B=32 -> 4/core)
  device : phase0  seq_qT / m2T / keyT_enc precompute (t-independent terms)
           phase1  127 sequential steps, fully column-form (features on
                   partitions, batch=4 moving dim; no transposes; sigmoid via
                   exp+reciprocal so only one act-table set is used)
           phase2  hoisted vocab projection (bf16) + log_softmax + DMA out
"""

import numpy as np

import concourse.bass as bass
import concourse.bacc as bacc
import concourse.mybir as mybir
import concourse.tile as tile
from concourse.bass_utils import run_bass_kernel_spmd

F32 = mybir.dt.float32
BF16 = mybir.dt.bfloat16
AF = mybir.ActivationFunctionType

B, L, D, T = 32, 512, 256, 128
H, NCOM, NB = 256, 8000, 128
V = NCOM + NB
NCORES = 8
BS = B // NCORES          # 4 examples per core
TT = T - 1                # 127 decode steps

# vocab chunking for phase 2: 15x512 + 1x320 common, then 128 batched
VCH = [(i * 512, 512) for i in range(15)] + [(7680, 320)]


def _pmajor(x, nchunk):
    """(nchunk*128, ...) -> (128, nchunk, ...) partition-major."""
    s = x.shape
    return np.ascontiguousarray(
        x.reshape(nchunk, 128, *s[1:]).transpose(1, 0, *range(2, 1 + len(s)))
    )


def _build(cfg):
    """Build the single-core program (SPMD-replicated across 8 cores)."""
    nc = bacc.Bacc("TRN2", target_bir_lowering=False, debug=False)

    dr = {}

    def din(name, shape, dt=F32):
        dr[name] = nc.dram_tensor(name, list(shape), dt, kind="ExternalInput").ap()
        return dr[name]

    enc_r = din("enc_r", (128, BS, 4, D), BF16)    # enc[b, 128*lc+p, d]
    enct = din("enct", (128, 2, BS, L), BF16)      # enc[b, l, 128*dc+p]
    tgtt = din("tgtt", (128, 2, TT, BS), BF16)     # tgt[b, t, 128*dc+p]
    cembt = din("cembt", (128, 2, NCOM), BF16)     # common[v, 128*dc+p]
    obt = din("obt", (128, 2, BS, NB), BF16)       # batched[b, v, 128*dc+p]
    wqt = din("wqt", (128, 2, D), BF16)                  # Wq[e, 128*dc+p]
    wket = din("wket", (128, 2, D), BF16)
    wkh2 = din("wkh2", (128, 2, D), BF16)   # akw[e, 256+f] with e on partitions
    cwt = din("cwt", (128, 4, H), BF16)                  # combine_w[g, 128*fc+p]
    lwt = din("lwt", (128, 4, 4 * H), BF16)              # [W_ih.T ; W_hh.T], ifog->ifog perm
    owt = din("owt", (128, 2, D), BF16)
    onesq = din("onesq", (128, 128), BF16)
    if cfg["any_bias"]:
        bqc = din("bqc", (128, 2))
        bkc = din("bkc", (128, 2))
        brow = din("brow", (1, 2 * H + 4 * H + D), BF16)   # [bc(256), bl(1024, ifog-perm), bo(256)]
        ones = din("ones", (1, BS), BF16)
    if cfg["enc_mask"]:
        emadd = din("emadd", (128, BS, 4))
    if cfg["out_mask"]:
        bmr = din("bmr", (BS, NB), BF16)
        onest = din("onest", (1, TT), BF16)

    out = nc.dram_tensor("out", [BS, TT, V], F32, kind="ExternalOutput").ap()

    with tile.TileContext(nc) as tc:
        with (
            tc.tile_pool(name="const", bufs=1) as kc,
            tc.tile_pool(name="state", bufs=3) as stp,
        ):
            # ---- persistent SBUF loads ----
            enc_sb = kc.tile([128, BS, 4, D], BF16)
            nc.sync.dma_start(enc_sb[:], enc_r[:])
            tgtt_sb = kc.tile([128, 2, TT, BS], BF16)
            nc.sync.dma_start(tgtt_sb[:], tgtt[:])
            wqt_sb = kc.tile([128, 2, D], BF16)
            nc.sync.dma_start(wqt_sb[:], wqt[:])
            wket_sb = kc.tile([128, 2, D], BF16)
            nc.sync.dma_start(wket_sb[:], wket[:])
            wkh2_sb = kc.tile([128, 2, D], BF16)
            nc.sync.dma_start(wkh2_sb[:], wkh2[:])
            cwt_sb = kc.tile([128, 4, H], BF16)
            nc.sync.dma_start(cwt_sb[:], cwt[:])
            lwt_sb = kc.tile([128, 4, 4 * H], BF16)
            nc.sync.dma_start(lwt_sb[:], lwt[:])
            owt_sb = kc.tile([128, 2, D], BF16)
            nc.sync.dma_start(owt_sb[:], owt[:])
            onesq_sb = kc.tile([128, 128], BF16)
            nc.sync.dma_start(onesq_sb[:], onesq[:])
            if cfg["any_bias"]:
                bqc_sb = kc.tile([128, 2], F32)
                nc.sync.dma_start(bqc_sb[:], bqc[:])
                bkc_sb = kc.tile([128, 2], F32)
                nc.sync.dma_start(bkc_sb[:], bkc[:])
                brow_sb = kc.tile([1, 2 * H + 4 * H + D], BF16)
                nc.sync.dma_start(brow_sb[:], brow[:])
                ones_sb = kc.tile([1, BS], BF16)
                nc.sync.dma_start(ones_sb[:], ones[:])
            if cfg["enc_mask"]:
                emadd_sb = kc.tile([128, BS, 4], F32)
                nc.sync.dma_start(emadd_sb[:], emadd[:])
            if cfg["out_mask"]:
                bmr_sb = kc.tile([BS, NB], BF16)
                nc.sync.dma_start(bmr_sb[:], bmr[:])
                onest_sb = kc.tile([1, TT], BF16)
                nc.sync.dma_start(onest_sb[:], onest[:])

            # phase-2-only tensors: issue their (large) DMAs last so they
            # don't delay the loads that gate phase 0/1 startup
            cembt_sb = kc.tile([128, 2, NCOM], BF16)
            nc.sync.dma_start(cembt_sb[:], cembt[:])
            obt_sb = kc.tile([128, 2, BS, NB], BF16)
            nc.sync.dma_start(obt_sb[:], obt[:])

            seqqt_sb = kc.tile([128, 2, BS, L], BF16)
            m2t_sb = kc.tile([128, 2, BS, L], BF16)
            ket_sb = kc.tile([128, 2, TT, BS], BF16)
            linT_sb = kc.tile([128, 2, BS, TT], BF16)

            # ---- phase 0: seq_qT, m2T, keyT_enc ----
            with (
                tc.tile_pool(name="enctp", bufs=1) as ep,
                tc.tile_pool(name="p0ps", bufs=2, space="PSUM") as p0,
            ):
                enct_sb = ep.tile([128, 2, BS, L], BF16)
                nc.sync.dma_start(enct_sb[:], enct[:])
                for b in range(BS):
                    for c in range(2):
                        ps = p0.tile([128, 512], F32)
                        for k in range(2):
                            nc.tensor.matmul(
                                ps[:],
                                wqt_sb[:, k, c * 128:(c + 1) * 128],
                                enct_sb[:, k, b, :],
                                start=(k == 0), stop=(k == 1),
                            )
                        if cfg["any_bias"]:
                            nc.scalar.activation(
                                seqqt_sb[:, c, b, :], ps[:], AF.Identity,
                                bias=bqc_sb[:, c:c + 1],
                            )
                        else:
                            nc.vector.tensor_copy(seqqt_sb[:, c, b, :], ps[:])
                for b in range(BS):
                    for c in range(2):
                        ps = p0.tile([128, 512], F32)
                        for k in range(2):
                            nc.tensor.matmul(
                                ps[:],
                                wkh2_sb[:, k, c * 128:(c + 1) * 128],
                                seqqt_sb[:, k, b, :],
                                start=(k == 0), stop=(k == 1),
                            )
                        nc.vector.tensor_copy(m2t_sb[:, c, b, :], ps[:])
                for c in range(2):
                    ps = p0.tile([128, 512], F32)
                    for k in range(2):
                        nc.tensor.matmul(
                            ps[:, 0:TT * BS],
                            wket_sb[:, k, c * 128:(c + 1) * 128],
                            tgtt_sb[:, k, :, :],
                            start=(k == 0), stop=(k == 1),
                        )
                    if cfg["any_bias"]:
                        nc.scalar.activation(
                            ket_sb[:, c, :, :], ps[:, 0:TT * BS], AF.Identity,
                            bias=bkc_sb[:, c:c + 1],
                        )
                    else:
                        nc.vector.tensor_copy(ket_sb[:, c, :, :], ps[:, 0:TT * BS])

            # ---- phase 1: 127 sequential steps, column form ----
            nb_ = 1 if cfg["any_bias"] else 0
            with (
                tc.tile_pool(name="scps", bufs=2, space="PSUM") as scps,
                tc.tile_pool(name="gpsp", bufs=1, space="PSUM") as gpsp,
                tc.tile_pool(name="smps", bufs=1, space="PSUM") as smps,
                tc.tile_pool(name="misc", bufs=1, space="PSUM") as mps,
                tc.tile_pool(name="work", bufs=3) as sbw,
            ):
                hT_cur = None
                c_cur = None

                def lin_mms(hT):
                    lps = mps.tile([128, 2, BS], F32, tag="lps")
                    for c in range(2):
                        for k in range(2):
                            nc.tensor.matmul(
                                lps[:, c, :],
                                owt_sb[:, k, c * 128:(c + 1) * 128],
                                hT[:, k, :],
                                start=(k == 0), stop=(k == 1 and nb_ == 0),
                            )
                        if nb_:
                            nc.tensor.matmul(
                                lps[:, c, :],
                                brow_sb[0:1, 5 * H + c * 128:5 * H + (c + 1) * 128],
                                ones_sb[0:1, :],
                                start=False, stop=True,
                            )
                    return lps

                for t in range(0 if cfg.get("skip_p1") else TT):
                    # -- scores (psum col [b, lc]): enc half has no h dep --
                    scp = scps.tile([128, BS, 4], F32, tag="scp")
                    for b in range(BS):
                        for lc in range(4):
                            col = scp[:, b, lc:lc + 1]
                            for k in range(2):
                                nc.tensor.matmul(
                                    col,
                                    seqqt_sb[:, k, b, lc * 128:(lc + 1) * 128],
                                    ket_sb[:, k, t, b:b + 1],
                                    start=(k == 0),
                                    stop=(t == 0 and k == 1),
                                )
                    if t > 0:
                        for b in range(BS):
                            for lc in range(4):
                                col = scp[:, b, lc:lc + 1]
                                for k in range(2):
                                    nc.tensor.matmul(
                                        col,
                                        m2t_sb[:, k, b, lc * 128:(lc + 1) * 128],
                                        hT_cur[:, k, b:b + 1],
                                        start=False, stop=(k == 1),
                                    )
                    # gates h-half early (off critical path)
                    gp = gpsp.tile([128, 8, BS], F32, tag="gp")
                    if t > 0:
                        for g in range(8):
                            for k in range(2):
                                nc.tensor.matmul(
                                    gp[:, g, :],
                                    lwt_sb[:, 2 + k, g * 128:(g + 1) * 128],
                                    hT_cur[:, k, :],
                                    start=(k == 0), stop=False,
                                )
                        # output projection for step t-1 (h_{t-1} ready now)
                        lps = lin_mms(hT_cur)
                        nc.vector.tensor_copy(linT_sb[:, :, :, t - 1], lps[:])

                    if cfg["enc_mask"]:
                        nc.vector.tensor_add(scp[:], scp[:], emadd_sb[:])

                    # -- softmax: exp -> (sums via ones-matmul) -> recip --
                    wt = sbw.tile([128, BS, 4], BF16, tag="wt")
                    nc.scalar.activation(wt[:], scp[:], AF.Exp)
                    smp = smps.tile([128, BS, 4], F32, tag="smp")
                    nc.tensor.matmul(smp[:], onesq_sb[:], wt[:], start=True, stop=True)
                    # attention context, column form
                    atp = mps.tile([128, 2, BS], F32, tag="atp")
                    for b in range(BS):
                        for dc in range(2):
                            for lc in range(4):
                                nc.tensor.matmul(
                                    atp[:, dc, b:b + 1],
                                    enc_sb[:, b, lc, dc * 128:(dc + 1) * 128],
                                    wt[:, b, lc:lc + 1],
                                    start=(lc == 0), stop=(lc == 3),
                                )
                    ssum = sbw.tile([128, BS, 1], F32, tag="ssum")
                    nc.vector.reduce_sum(ssum[:], smp[:], axis=mybir.AxisListType.X)
                    rb = sbw.tile([128, BS], F32, tag="rb")
                    nc.vector.reciprocal(rb[:], ssum[:, :, 0])
                    ats = sbw.tile([128, 2, BS], BF16, tag="ats")
                    for dc in range(2):
                        nc.vector.tensor_mul(ats[:, dc, :], atp[:, dc, :], rb[:])

                    # -- combine + relu --
                    cbp = mps.tile([128, 2, BS], F32, tag="cbp")
                    cl = [tgtt_sb[:, 0, t, :], tgtt_sb[:, 1, t, :],
                          ats[:, 0, :], ats[:, 1, :]]
                    for fc in range(2):
                        for k in range(4):
                            nc.tensor.matmul(
                                cbp[:, fc, :],
                                cwt_sb[:, k, fc * 128:(fc + 1) * 128],
                                cl[k],
                                start=(k == 0), stop=(k == 3 and nb_ == 0),
                            )
                        if nb_:
                            nc.tensor.matmul(
                                cbp[:, fc, :],
                                brow_sb[0:1, fc * 128:(fc + 1) * 128],
                                ones_sb[0:1, :],
                                start=False, stop=True,
                            )
                    cbT = sbw.tile([128, 2, BS], BF16, tag="cbT")
                    nc.scalar.activation(cbT[:], cbp[:], AF.Relu)

                    # -- LSTM gates, comb half (accumulates into gp) --
                    for g in range(8):
                        for k in range(2):
                            nc.tensor.matmul(
                                gp[:, g, :],
                                lwt_sb[:, k, g * 128:(g + 1) * 128],
                                cbT[:, k, :],
                                start=(t == 0 and k == 0),
                                stop=(k == 1 and nb_ == 0),
                            )
                        if nb_:
                            nc.tensor.matmul(
                                gp[:, g, :],
                                brow_sb[0:1, 2 * H + g * 128:2 * H + (g + 1) * 128],
                                ones_sb[0:1, :],
                                start=False, stop=True,
                            )

                    # -- gate nonlinearities: sigmoid(x) = 1/(1+exp(-x)) --
                    sie = sbw.tile([128, 6, BS], F32, tag="sie")
                    nc.scalar.activation(sie[:], gp[:, 0:6, :], AF.Exp, scale=-1.0)
                    tg = sbw.tile([128, 2, BS], F32, tag="tg")
                    nc.scalar.activation(tg[:], gp[:, 6:8, :], AF.Tanh)
                    si = sbw.tile([128, 6, BS], F32, tag="si")
                    nc.vector.tensor_scalar_add(si[:], sie[:], 1.0)
                    nc.vector.reciprocal(si[:], si[:])

                    # -- c/h update --
                    m1 = stp.tile([128, 2, BS], F32, tag="cstate")
                    nc.vector.tensor_mul(m1[:], si[:, 0:2, :], tg[:])
                    if t > 0:
                        m2 = sbw.tile([128, 2, BS], F32, tag="m2")
                        nc.vector.tensor_mul(m2[:], si[:, 2:4, :], c_cur[:])
                        c_new = stp.tile([128, 2, BS], F32, tag="cstate")
                        nc.vector.tensor_add(c_new[:], m1[:], m2[:])
                    else:
                        c_new = m1
                    tc_ = sbw.tile([128, 2, BS], F32, tag="tc")
                    nc.scalar.activation(tc_[:], c_new[:], AF.Tanh)
                    hT_new = stp.tile([128, 2, BS], BF16, tag="hstate")
                    nc.vector.tensor_mul(hT_new[:], si[:, 4:6, :], tc_[:])

                    hT_cur, c_cur = hT_new, c_new

                if not cfg.get("skip_p1"):
                    lps = lin_mms(hT_cur)
                    nc.vector.tensor_copy(linT_sb[:, :, :, TT - 1], lps[:])

            # ---- phase 2: vocab projection + log_softmax ----
            with (
                tc.tile_pool(name="p2ps", bufs=4, space="PSUM") as p2,
                tc.tile_pool(name="ep2", bufs=1) as ep2,
                tc.tile_pool(name="outst", bufs=3) as osp,
                tc.tile_pool(name="sm2", bufs=2) as sm2,
            ):
                nch = len(VCH) + 1
                for b in range(0 if cfg.get("skip_p2") else BS):
                    et = ep2.tile([TT, nch, 512], F32, tag="et")
                    ss = sm2.tile([TT, nch], F32, tag="ss")
                    for j in range(nch):
                        if j < len(VCH):
                            off, w = VCH[j]
                            rhs = [cembt_sb[:, k, off:off + w] for k in range(2)]
                        else:
                            w = NB
                            rhs = [obt_sb[:, k, b, :] for k in range(2)]
                        ps = p2.tile([TT, 512], F32, tag="p2")
                        for k in range(2):
                            nc.tensor.matmul(
                                ps[:, 0:w],
                                linT_sb[:, k, b, :],
                                rhs[k],
                                start=(k == 0),
                                stop=(k == 1 and not (j == nch - 1 and cfg["out_mask"])),
                            )
                        if j == nch - 1 and cfg["out_mask"]:
                            nc.tensor.matmul(
                                ps[:, 0:w], onest_sb[0:1, :],
                                bmr_sb[b:b + 1, :],
                                start=False, stop=True,
                            )
                        nc.scalar.activation(
                            et[:, j, 0:w], ps[:, 0:w], AF.Exp,
                            accum_out=ss[:, j:j + 1],
                        )
                    st = sm2.tile([TT, 1], F32, tag="st")
                    nc.vector.reduce_sum(
                        st[:], ss[:], axis=mybir.AxisListType.X
                    )
                    rt = sm2.tile([TT, 1], F32, tag="rt")
                    nc.vector.reciprocal(rt[:], st[:])
                    for j in range(nch):
                        if j < len(VCH):
                            off, w = VCH[j]
                            voff = off
                        else:
                            w, voff = NB, NCOM
                        ot = osp.tile([TT, 512], F32, tag="ot")
                        nc.scalar.activation(
                            ot[:, 0:w], et[:, j, 0:w], AF.Ln, scale=rt[:],
                        )
                        nc.sync.dma_start(
                            out[b, :, voff:voff + w], ot[:, 0:w]
                        )

    nc.compile()
    return nc


_CACHE = {}


def kernel(**inputs):
    inp = {k: np.asarray(v) for k, v in inputs.items()}
    enc = inp["encoder_outputs"].astype(np.float32)
    encm = inp["encoder_outputs_mask"]
    ob = inp["output_batched_encodings"].astype(np.float32)
    obm = inp["output_batched_encodings_mask"]
    idx = inp["target_idxs"]
    cem = inp["common_embedding"].astype(np.float32)
    akw = inp["attn_key_w"].astype(np.float32)
    akb = inp["attn_key_b"].astype(np.float32)
    aqw = inp["attn_query_w"].astype(np.float32)
    aqb = inp["attn_query_b"].astype(np.float32)
    cw = inp["combine_w"].astype(np.float32)
    cb = inp["combine_b"].astype(np.float32)
    wih = inp["lstm_w_ih"].astype(np.float32)
    whh = inp["lstm_w_hh"].astype(np.float32)
    bih = inp["lstm_b_ih"].astype(np.float32)
    bhh = inp["lstm_b_hh"].astype(np.float32)
    ow = inp["out_w"].astype(np.float32)
    obias = inp["out_b"].astype(np.float32)

    # teacher-forced embedding gather (host: data-dependent indexing)
    is_c = idx < NCOM
    cidx = np.clip(idx, 0, NCOM - 1)
    bidx = np.clip(idx - NCOM, 0, NB - 1)
    ge_c = cem[cidx]                                   # (B, T, D)
    ge_b = np.take_along_axis(ob, bidx[..., None], axis=1)
    tgt = np.where(is_c[..., None], ge_c, ge_b)[:, :TT, :].astype(np.float32)

    any_bias = bool(
        np.any(akb) or np.any(aqb) or np.any(cb) or np.any(bih)
        or np.any(bhh) or np.any(obias)
    )
    enc_mask = not bool(encm.all())
    out_mask = not bool(obm.all())

    cfg = {"any_bias": any_bias, "enc_mask": enc_mask, "out_mask": out_mask}
    key = (any_bias, enc_mask, out_mask)
    if key not in _CACHE:
        _CACHE[key] = _build(cfg)
    nc = _CACHE[key]

    # lstm weights, gate order [i, f, o, g] (so one Act covers i,f,o)
    lcat = np.concatenate([wih.T, whh.T], axis=0)       # (2*H(g), 4*H)
    perm = np.concatenate(
        [lcat[:, 0:H], lcat[:, H:2 * H], lcat[:, 3 * H:4 * H], lcat[:, 2 * H:3 * H]],
        axis=1,
    )

    # shared (replicated) tensors
    import ml_dtypes
    bft = ml_dtypes.bfloat16
    shared = {
        "cembt": _pmajor(np.ascontiguousarray(cem.T), 2).astype(bft),
        "wqt": _pmajor(np.ascontiguousarray(aqw.T), 2).astype(bft),
        "wket": _pmajor(np.ascontiguousarray(akw[:, :D].T), 2).astype(bft),
        "wkh2": _pmajor(np.ascontiguousarray(akw[:, D:]), 2).astype(bft),
        "cwt": _pmajor(np.ascontiguousarray(cw.T), 4).astype(bft),
        "lwt": _pmajor(np.ascontiguousarray(perm), 4).astype(bft),
        "owt": _pmajor(np.ascontiguousarray(ow.T), 2).astype(bft),
        "onesq": np.ones((128, 128), bft),
    }
    if any_bias:
        bl = bih + bhh
        blp = np.concatenate(
            [bl[0:H], bl[H:2 * H], bl[3 * H:4 * H], bl[2 * H:3 * H]]
        )
        shared["bqc"] = _pmajor(aqb, 2)
        shared["bkc"] = _pmajor(akb, 2)
        shared["brow"] = np.concatenate(
            [cb, blp, obias]
        )[None, :].astype(bft)
        shared["ones"] = np.ones((1, BS), bft)
    if out_mask:
        shared["onest"] = np.ones((1, TT), bft)

    in_maps = []
    for c in range(NCORES):
        sl = slice(c * BS, (c + 1) * BS)
        e = enc[sl]                                    # (BS, L, D)
        tg_ = tgt[sl]                                  # (BS, TT, D)
        obs = ob[sl]                                   # (BS, NB, D)
        m = dict(shared)
        m["enc_r"] = np.ascontiguousarray(
            e.reshape(BS, 4, 128, D).transpose(2, 0, 1, 3)
        ).astype(bft)
        m["enct"] = np.ascontiguousarray(
            e.transpose(2, 0, 1).reshape(2, 128, BS, L).transpose(1, 0, 2, 3)
        ).astype(bft)
        m["tgtt"] = np.ascontiguousarray(
            tg_.transpose(2, 1, 0).reshape(2, 128, TT, BS).transpose(1, 0, 2, 3)
        ).astype(bft)
        m["obt"] = np.ascontiguousarray(
            obs.transpose(2, 0, 1).reshape(2, 128, BS, NB).transpose(1, 0, 2, 3)
        ).astype(ml_dtypes.bfloat16)
        if enc_mask:
            em = np.where(encm[sl], 0.0, -1e30).astype(np.float32)  # (BS, L)
            m["emadd"] = np.ascontiguousarray(
                em.reshape(BS, 4, 128).transpose(2, 0, 1)
            )
        if out_mask:
            m["bmr"] = np.where(obm[sl], 0.0, -1e30).astype(bft)
        in_maps.append(m)

    res = run_bass_kernel_spmd(nc, in_maps, list(range(NCORES)))
    outs = [res.results[c]["out"].reshape(BS, TT, V) for c in range(NCORES)]
    return np.concatenate(outs, axis=0).astype(np.float32)
